# revision 5
# baseline (speedup 1.0000x reference)
"""DiceLoss kernel for Trainium2 (raw Bass, no Tile), 8-core data parallel.

Problem: predict/target [2, 4, 64, 256, 256] f32.
  p = sigmoid(predict); per (b, o, d) slice of 65536 elements:
    num = sum(p*t), den = sum(p) + sum(t) + 1
    dice = 1 - 2*num/den
  per-(b,o) mean over valid d slices, then mean over the 8 (b,o) pairs.

Sharding: B*O = 8 (b, o) pairs, one per core; each core handles its
pair's 64 depth slices (16 MiB predict + 16 MiB target -> DMA-bound,
~94 us/core at ~358 GB/s).

Per slice (viewed [128 x 512]) on each core:
  ScalarE: sigmoid(predict) -> sig, accum_out -> sum(p) column
           copy(target) -> scratch, accum_out -> sum(t) column
           (both functions live in one ACT table -> no reloads)
  VectorE: scalar_tensor_tensor (sig*1.0)*target -> scratch,
           accum_out -> sum(p*t) column   (single fused op)
(tensor_tensor_reduce is not used: its InstISA encoding is rejected by
this walrus build. Tile is not used: its kernel-tail drain exceeds this
build's per-instruction sync-wait limit.)

Scratch outputs rotate over 2 slots so same-engine WAW conflicts sit at
distance 2; a wait_ge on the engine's own (already-passed) semaphore
value proves the ordering to the race detector without stalling.

The [128, 64] accumulator tiles are DMA'd out; host does the partition
sum and the tiny dice math over 512 slices.
"""

from contextlib import ExitStack

import numpy as np

import concourse.bass as bass
from concourse import mybir
from concourse.bass_utils import run_bass_kernel_spmd

N_CORES = 8
B, O, D = 2, 4, 64
HW = 256 * 256          # elements per slice
P = 128                 # SBUF partitions
F = HW // P             # 512 free elems per partition per slice
S = (B * O * D) // N_CORES  # 64 slices per core (= one (b,o) pair)
G = 8                   # slices per DMA group (2 MiB per dma_start)
IN_BUFS = 3             # input group buffers (pr/tg)
SIG_BUFS = 2            # sigmoid output group buffers
SMOOTH = 1.0

f32 = mybir.dt.float32
AF = mybir.ActivationFunctionType
ALU = mybir.AluOpType


def build_nc(n_slices=S, group=G):
    """Build the per-core Bass program (same program on all cores)."""
    assert n_slices % group == 0
    ng = n_slices // group
    nc = bass.Bass("TRN2", debug=False, enable_asserts=False)

    pred = nc.dram_tensor("predict", [n_slices, P, F], f32, kind="ExternalInput").ap()
    tgt = nc.dram_tensor("target", [n_slices, P, F], f32, kind="ExternalInput").ap()
    out_spt = nc.dram_tensor("out_spt", [P, n_slices], f32, kind="ExternalOutput").ap()
    out_sp = nc.dram_tensor("out_sp", [P, n_slices], f32, kind="ExternalOutput").ap()
    out_st = nc.dram_tensor("out_st", [P, n_slices], f32, kind="ExternalOutput").ap()

    with ExitStack() as ctx:
        pr_buf = ctx.enter_context(nc.sbuf_tensor([P, IN_BUFS, group * F], f32))
        tg_buf = ctx.enter_context(nc.sbuf_tensor([P, IN_BUFS, group * F], f32))
        sig_buf = ctx.enter_context(nc.sbuf_tensor([P, SIG_BUFS, group * F], f32))
        scr_a = ctx.enter_context(nc.sbuf_tensor([P, 2, F], f32))  # copy dest
        scr_v = ctx.enter_context(nc.sbuf_tensor([P, 2, F], f32))  # stt dest
        sp_acc = ctx.enter_context(nc.sbuf_tensor([P, n_slices], f32))
        st_acc = ctx.enter_context(nc.sbuf_tensor([P, n_slices], f32))
        spt_acc = ctx.enter_context(nc.sbuf_tensor([P, n_slices], f32))
        # One DMA sem per input slot: at most one group's 2 loads in flight
        # per sem, so "sem >= 32*uses" proves both loads complete. A single
        # cumulative sem would be unsound with >2 DMAs in flight (fast SDMA
        # engines can contribute shards of later DMAs to the count).
        dma_sems = [
            ctx.enter_context(nc.semaphore(f"dma_sem{i}")) for i in range(IN_BUFS)
        ]
        out_sem = ctx.enter_context(nc.semaphore("out_sem"))
        act_sem = ctx.enter_context(nc.semaphore("act_sem"))  # +1 sigmoid, +1 copy
        dve_sem = ctx.enter_context(nc.semaphore("dve_sem"))  # +1 per stt
        block = ctx.enter_context(nc.Block())

        @block.sync
        def _(sync):
            for g in range(ng):
                slot = g % IN_BUFS
                if g >= IN_BUFS:
                    done = g - IN_BUFS + 1   # groups fully consumed
                    sync.wait_ge(act_sem, 2 * done * group)
                    sync.wait_ge(dve_sem, done * group)
                sync.dma_start(
                    pr_buf[:, slot, :].rearrange("p (s f) -> p s f", s=group),
                    pred[g * group:(g + 1) * group].rearrange("s p f -> p s f"),
                ).then_inc(dma_sems[slot], 16)
                sync.dma_start(
                    tg_buf[:, slot, :].rearrange("p (s f) -> p s f", s=group),
                    tgt[g * group:(g + 1) * group].rearrange("s p f -> p s f"),
                ).then_inc(dma_sems[slot], 16)
            sync.wait_ge(act_sem, 2 * n_slices)
            sync.wait_ge(dve_sem, n_slices)
            sync.dma_start(out_spt, spt_acc[:]).then_inc(out_sem, 16)
            sync.dma_start(out_sp, sp_acc[:]).then_inc(out_sem, 16)
            sync.dma_start(out_st, st_acc[:]).then_inc(out_sem, 16)
            sync.wait_ge(out_sem, 48)

        @block.scalar
        def _(scalar):
            for g in range(ng):
                slot = g % IN_BUFS
                sslot = g % SIG_BUFS
                scalar.wait_ge(dma_sems[slot], 32 * (g // IN_BUFS + 1))
                if g >= SIG_BUFS:
                    # DVE must be done reading sig of the group in this slot
                    scalar.wait_ge(dve_sem, (g - SIG_BUFS + 1) * group)
                for s in range(group):
                    q = g * group + s
                    c = slice(s * F, (s + 1) * F)
                    nc.scalar.activation(
                        sig_buf[:, sslot, c], pr_buf[:, slot, c], AF.Sigmoid,
                        accum_out=sp_acc[:, q:q + 1],
                    ).then_inc(act_sem, 1)
                    if q >= 2:
                        # scr_a slot WAW vs copy(q-2); already satisfied
                        scalar.wait_ge(act_sem, 2 * (q - 1))
                    nc.scalar.activation(
                        scr_a[:, q % 2, :], tg_buf[:, slot, c], AF.Copy,
                        accum_out=st_acc[:, q:q + 1],
                    ).then_inc(act_sem, 1)

        @block.vector
        def _(vector):
            for g in range(ng):
                slot = g % IN_BUFS
                sslot = g % SIG_BUFS
                for s in range(group):
                    q = g * group + s
                    c = slice(s * F, (s + 1) * F)
                    vector.wait_ge(act_sem, 2 * q + 1)   # sigmoid(q) done
                    if q >= 2:
                        # scr_v slot WAW vs stt(q-2); already satisfied
                        vector.wait_ge(dve_sem, q - 1)
                    nc.vector.scalar_tensor_tensor(
                        out=scr_v[:, q % 2, :],
                        in0=sig_buf[:, sslot, c],
                        scalar=1.0,
                        in1=tg_buf[:, slot, c],
                        op0=ALU.mult, op1=ALU.mult,
                        accum_out=spt_acc[:, q:q + 1],
                    ).then_inc(dve_sem, 1)

    return nc


_NC_CACHE = {}


def _get_nc():
    if "nc" not in _NC_CACHE:
        _NC_CACHE["nc"] = build_nc()
    return _NC_CACHE["nc"]


def kernel(predict: np.ndarray, target: np.ndarray) -> np.ndarray:
    assert predict.shape == (B, O, D, 256, 256)
    pred_sh = np.ascontiguousarray(predict, dtype=np.float32).reshape(N_CORES, S, P, F)
    tgt_sh = np.ascontiguousarray(target, dtype=np.float32).reshape(N_CORES, S, P, F)
    in_maps = [
        {"predict": pred_sh[i], "target": tgt_sh[i]}
        for i in range(N_CORES)
    ]
    nc = _get_nc()
    res = run_bass_kernel_spmd(nc, in_maps, list(range(N_CORES)))

    # Host-side finish: partition-sum the [128, S] partials, then the tiny
    # dice math over 512 slices.
    spt = np.empty((N_CORES, S), np.float64)
    sp = np.empty((N_CORES, S), np.float64)
    st = np.empty((N_CORES, S), np.float64)
    for i, r in enumerate(res.results):
        spt[i] = r["out_spt"].astype(np.float64).sum(axis=0)
        sp[i] = r["out_sp"].astype(np.float64).sum(axis=0)
        st[i] = r["out_st"].astype(np.float64).sum(axis=0)

    dice = 1.0 - 2.0 * spt / (sp + st + SMOOTH)          # [B*O, D]
    tfirst = target.reshape(B * O, D, HW)[:, :, 0]       # [B*O, D]
    valid = (tfirst != -1.0).astype(np.float64)
    per_pair = (dice * valid).sum(axis=-1) / valid.sum(axis=-1)  # [B*O]
    return np.array(per_pair.mean(), dtype=np.float32)


# revision 14
# speedup vs baseline: 7.3794x; 7.3794x over previous
"""DiceLoss kernel for Trainium2 (raw Bass, no Tile), 8-core data parallel.

Problem: predict/target [2, 4, 64, 256, 256] f32.
  p = sigmoid(predict); per (b, o, d) slice of 65536 elements:
    num = sum(p*t), den = sum(p) + sum(t) + 1
    dice = 1 - 2*num/den
  per-(b,o) mean over valid d slices, then mean over the 8 (b,o) pairs.

Sharding: B*O = 8 (b, o) pairs, one per core; each core handles its
pair's 64 depth slices (16 MiB predict + 16 MiB target -> DMA-bound,
~94 us/core at ~358 GB/s). Host stacks predict+target per core into one
[2, 64, 128, 512] array so each group load is a single 4 MiB dma_start.

Per slice (viewed [128 x 512]) on each core:
  ScalarE: sigmoid(predict) -> sig, accum_out -> sum(p) column
           copy(target) -> scratch, accum_out -> sum(t) column
           (both functions live in one ACT table -> no reloads)
  VectorE: scalar_tensor_tensor (sig*1.0)*target -> scratch,
           accum_out -> sum(p*t) column   (single fused op)
(tensor_tensor_reduce is not used: its InstISA encoding is rejected by
this walrus build. Tile is not used: its kernel-tail drain exceeds this
build's per-instruction sync-wait limit.)

Scratch outputs rotate over 2 slots so same-engine WAW conflicts sit at
distance 2; a wait_ge on the engine's own (already-passed) semaphore
value proves the ordering to the race detector without stalling.

The [128, 3, 64] accumulator tile is DMA'd out once; host does the
partition sum and the tiny dice math over 512 slices.
"""

from contextlib import ExitStack

import numpy as np

import concourse.bass as bass
from concourse import mybir
from concourse.bass_utils import run_bass_kernel_spmd

N_CORES = 8
B, O, D = 2, 4, 64
HW = 256 * 256          # elements per slice
P = 128                 # SBUF partitions
F = HW // P             # 512 free elems per partition per slice
S = (B * O * D) // N_CORES  # 64 slices per core (= one (b,o) pair)
G = 8                   # slices per DMA group (4 MiB per dma_start)
IN_BUFS = 3             # input group buffers
SIG_BUFS = 2            # sigmoid output group buffers
SMOOTH = 1.0

f32 = mybir.dt.float32
AF = mybir.ActivationFunctionType
ALU = mybir.AluOpType


def build_nc(n_slices=S, group=G, repeats=1):
    """Build the per-core Bass program (same program on all cores).

    repeats > 1 re-runs the whole body that many times (re-reading the
    same DRAM) — used only for slope-based wall-clock timing."""
    assert n_slices % group == 0
    ng = n_slices // group
    total_groups = ng * repeats
    nc = bass.Bass("TRN2", debug=False, enable_asserts=False)

    # inp[s, 0] = predict slice s, inp[s, 1] = target slice s (host-stacked);
    # the (s, 2) dims merge into one stride run so a group load is a 3-dim AP.
    inp = nc.dram_tensor("inp", [n_slices, 2, P, F], f32, kind="ExternalInput").ap()
    # out_acc[:, 0] = sum(p), out_acc[:, 1] = sum(t), out_acc[:, 2] = sum(p*t)
    out_acc = nc.dram_tensor("out_acc", [P, 3, n_slices], f32,
                             kind="ExternalOutput").ap()

    with ExitStack() as ctx:
        # in_buf slot layout: m = 2*s + j blocks of F: predict slice s at
        # m=2s, target slice s at m=2s+1
        in_buf = ctx.enter_context(nc.sbuf_tensor([P, IN_BUFS, 2 * group * F], f32))
        sig_buf = ctx.enter_context(nc.sbuf_tensor([P, SIG_BUFS, group * F], f32))
        scr_a = ctx.enter_context(nc.sbuf_tensor([P, 2, F], f32))  # copy dest
        scr_v = ctx.enter_context(nc.sbuf_tensor([P, 2, F], f32))  # stt dest
        acc = ctx.enter_context(nc.sbuf_tensor([P, 3, n_slices], f32))
        # One DMA sem per input slot: at most one group's load in flight per
        # sem, so "sem >= 16*uses" proves the load is complete. A single
        # cumulative sem would be unsound with >1 DMA in flight (fast SDMA
        # engines can contribute shards of later DMAs to the count).
        dma_sems = [
            ctx.enter_context(nc.semaphore(f"dma_sem{i}")) for i in range(IN_BUFS)
        ]
        out_sem = ctx.enter_context(nc.semaphore("out_sem"))
        act_sem = ctx.enter_context(nc.semaphore("act_sem"))  # +1 sigmoid, +1 copy
        dve_sem = ctx.enter_context(nc.semaphore("dve_sem"))  # +1 per stt
        block = ctx.enter_context(nc.Block())

        sp_acc = acc[:, 0, :]
        st_acc = acc[:, 1, :]
        spt_acc = acc[:, 2, :]

        @block.sync
        def _(sync):
            for t in range(total_groups):
                g = t % ng
                slot = t % IN_BUFS
                if t >= IN_BUFS:
                    done = t - IN_BUFS + 1   # groups fully consumed
                    sync.wait_ge(act_sem, 2 * done * group)
                    sync.wait_ge(dve_sem, done * group)
                sync.dma_start(
                    in_buf[:, slot, :].rearrange("p (m f) -> p m f", f=F),
                    inp[g * group:(g + 1) * group].rearrange("s j p f -> p (s j) f"),
                ).then_inc(dma_sems[slot], 16)
            sync.wait_ge(act_sem, 2 * total_groups * group)
            sync.wait_ge(dve_sem, total_groups * group)
            sync.dma_start(out_acc, acc[:]).then_inc(out_sem, 16)
            sync.wait_ge(out_sem, 16)

        @block.scalar
        def _(scalar):
            for t in range(total_groups):
                g = t % ng
                slot = t % IN_BUFS
                sslot = t % SIG_BUFS
                scalar.wait_ge(dma_sems[slot], 16 * (t // IN_BUFS + 1))
                if t >= SIG_BUFS:
                    # DVE must be done reading sig of the group in this slot
                    scalar.wait_ge(dve_sem, (t - SIG_BUFS + 1) * group)
                for s in range(group):
                    tq = t * group + s       # absolute op counter
                    q = g * group + s        # data/accum column
                    c = slice(s * F, (s + 1) * F)
                    cp = slice((2 * s) * F, (2 * s + 1) * F)      # predict
                    ct = slice((2 * s + 1) * F, (2 * s + 2) * F)  # target
                    nc.scalar.activation(
                        sig_buf[:, sslot, c], in_buf[:, slot, cp], AF.Sigmoid,
                        accum_out=sp_acc[:, q:q + 1],
                    ).then_inc(act_sem, 1)
                    if tq >= 2:
                        # scr_a slot WAW vs copy(tq-2); already satisfied
                        scalar.wait_ge(act_sem, 2 * (tq - 1))
                    nc.scalar.activation(
                        scr_a[:, tq % 2, :], in_buf[:, slot, ct], AF.Copy,
                        accum_out=st_acc[:, q:q + 1],
                    ).then_inc(act_sem, 1)

        @block.vector
        def _(vector):
            for t in range(total_groups):
                g = t % ng
                slot = t % IN_BUFS
                sslot = t % SIG_BUFS
                for s in range(group):
                    tq = t * group + s
                    q = g * group + s
                    c = slice(s * F, (s + 1) * F)
                    ct = slice((2 * s + 1) * F, (2 * s + 2) * F)  # target
                    vector.wait_ge(act_sem, 2 * tq + 1)   # sigmoid(tq) done
                    if tq >= 2:
                        # scr_v slot WAW vs stt(tq-2); already satisfied
                        vector.wait_ge(dve_sem, tq - 1)
                    nc.vector.scalar_tensor_tensor(
                        out=scr_v[:, tq % 2, :],
                        in0=sig_buf[:, sslot, c],
                        scalar=1.0,
                        in1=in_buf[:, slot, ct],
                        op0=ALU.mult, op1=ALU.mult,
                        accum_out=spt_acc[:, q:q + 1],
                    ).then_inc(dve_sem, 1)

    return nc


_NC_CACHE = {}


def _get_nc():
    if "nc" not in _NC_CACHE:
        _NC_CACHE["nc"] = build_nc()
    return _NC_CACHE["nc"]


def shard_inputs(predict, target):
    pred_sh = np.ascontiguousarray(predict, dtype=np.float32).reshape(
        N_CORES, S, P, F)
    tgt_sh = np.ascontiguousarray(target, dtype=np.float32).reshape(
        N_CORES, S, P, F)
    return [
        {"inp": np.stack([pred_sh[i], tgt_sh[i]], axis=1)}
        for i in range(N_CORES)
    ]


def finish(results, target):
    """Host-side: partition-sum [128, 3, S] partials + dice math."""
    sp = np.empty((N_CORES, S), np.float64)
    st = np.empty((N_CORES, S), np.float64)
    spt = np.empty((N_CORES, S), np.float64)
    for i, r in enumerate(results):
        a = r["out_acc"].astype(np.float64).sum(axis=0)   # [3, S]
        sp[i], st[i], spt[i] = a[0], a[1], a[2]

    dice = 1.0 - 2.0 * spt / (sp + st + SMOOTH)          # [B*O, D]
    tfirst = target.reshape(B * O, D, HW)[:, :, 0]       # [B*O, D]
    valid = (tfirst != -1.0).astype(np.float64)
    per_pair = (dice * valid).sum(axis=-1) / valid.sum(axis=-1)  # [B*O]
    return np.array(per_pair.mean(), dtype=np.float32)


def kernel(predict: np.ndarray, target: np.ndarray) -> np.ndarray:
    assert predict.shape == (B, O, D, 256, 256)
    in_maps = shard_inputs(predict, target)
    nc = _get_nc()
    res = run_bass_kernel_spmd(nc, in_maps, list(range(N_CORES)))
    return finish(res.results, target)


# revision 15
# speedup vs baseline: 7.5169x; 1.0186x over previous
"""DiceLoss kernel for Trainium2 (raw Bass, no Tile), 8-core data parallel.

Problem: predict/target [2, 4, 64, 256, 256] f32.
  p = sigmoid(predict); per (b, o, d) slice of 65536 elements:
    num = sum(p*t), den = sum(p) + sum(t) + 1
    dice = 1 - 2*num/den
  per-(b,o) mean over valid d slices, then mean over the 8 (b,o) pairs.

Sharding: B*O = 8 (b, o) pairs, one per core; each core handles its
pair's 64 depth slices (16 MiB predict + 16 MiB target -> DMA-bound,
~94 us/core at ~358 GB/s). Host stacks predict+target per core into one
[2, 64, 128, 512] array so each group load is a single 4 MiB dma_start.

Per slice (viewed [128 x 512]) on each core:
  ScalarE: sigmoid(predict) -> sig, accum_out -> sum(p) column
           copy(target) -> scratch, accum_out -> sum(t) column
           (both functions live in one ACT table -> no reloads)
  VectorE: scalar_tensor_tensor (sig*1.0)*target -> scratch,
           accum_out -> sum(p*t) column   (single fused op)
(tensor_tensor_reduce is not used: its InstISA encoding is rejected by
this walrus build. Tile is not used: its kernel-tail drain exceeds this
build's per-instruction sync-wait limit.)

Scratch outputs rotate over 2 slots so same-engine WAW conflicts sit at
distance 2; a wait_ge on the engine's own (already-passed) semaphore
value proves the ordering to the race detector without stalling.

The [128, 3, 64] accumulator tile is DMA'd out once; host does the
partition sum and the tiny dice math over 512 slices.
"""

from contextlib import ExitStack

import numpy as np

import concourse.bass as bass
from concourse import mybir
from concourse.bass_utils import run_bass_kernel_spmd

N_CORES = 8
B, O, D = 2, 4, 64
HW = 256 * 256          # elements per slice
P = 128                 # SBUF partitions
F = HW // P             # 512 free elems per partition per slice
S = (B * O * D) // N_CORES  # 64 slices per core (= one (b,o) pair)
G = 8                   # slices per DMA group (4 MiB per dma_start)
IN_BUFS = 3             # input group buffers
SIG_BUFS = 2            # sigmoid output group buffers
SMOOTH = 1.0

f32 = mybir.dt.float32
AF = mybir.ActivationFunctionType
ALU = mybir.AluOpType


def build_nc(n_slices=S, group=G, repeats=1):
    """Build the per-core Bass program (same program on all cores).

    repeats > 1 re-runs the whole body that many times (re-reading the
    same DRAM) — used only for slope-based wall-clock timing."""
    assert n_slices % group == 0
    ng = n_slices // group
    total_groups = ng * repeats
    nc = bass.Bass("TRN2", debug=False, enable_asserts=False)

    # inp[s, 0] = predict slice s, inp[s, 1] = target slice s (host-stacked);
    # the (s, 2) dims merge into one stride run so a group load is a 3-dim AP.
    inp = nc.dram_tensor("inp", [n_slices, 2, P, F], f32, kind="ExternalInput").ap()
    # out_acc[:, 0] = sum(p), out_acc[:, 1] = sum(t), out_acc[:, 2] = sum(p*t)
    out_acc = nc.dram_tensor("out_acc", [P, 3, n_slices], f32,
                             kind="ExternalOutput").ap()

    with ExitStack() as ctx:
        # in_buf slot layout: m = 2*s + j blocks of F: predict slice s at
        # m=2s, target slice s at m=2s+1
        in_buf = ctx.enter_context(nc.sbuf_tensor([P, IN_BUFS, 2 * group * F], f32))
        sig_buf = ctx.enter_context(nc.sbuf_tensor([P, SIG_BUFS, group * F], f32))
        scr_a = ctx.enter_context(nc.sbuf_tensor([P, 2, F], f32))  # copy dest
        scr_v = ctx.enter_context(nc.sbuf_tensor([P, 2, F], f32))  # stt dest
        acc = ctx.enter_context(nc.sbuf_tensor([P, 3, n_slices], f32))
        # One DMA sem per input slot: at most one group's load in flight per
        # sem, so "sem >= 16*uses" proves the load is complete. A single
        # cumulative sem would be unsound with >1 DMA in flight (fast SDMA
        # engines can contribute shards of later DMAs to the count).
        dma_sems = [
            ctx.enter_context(nc.semaphore(f"dma_sem{i}")) for i in range(IN_BUFS)
        ]
        out_sem = ctx.enter_context(nc.semaphore("out_sem"))
        act_sem = ctx.enter_context(nc.semaphore("act_sem"))  # +1 sigmoid, +1 copy
        dve_sem = ctx.enter_context(nc.semaphore("dve_sem"))  # +1 per stt
        block = ctx.enter_context(nc.Block())

        sp_acc = acc[:, 0, :]
        st_acc = acc[:, 1, :]
        spt_acc = acc[:, 2, :]

        @block.sync
        def _(sync):
            for t in range(total_groups):
                g = t % ng
                slot = t % IN_BUFS
                if t >= IN_BUFS:
                    done = t - IN_BUFS + 1   # groups fully consumed
                    sync.wait_ge(act_sem, 2 * done * group)
                    sync.wait_ge(dve_sem, done * group)
                sync.dma_start(
                    in_buf[:, slot, :].rearrange("p (m f) -> p m f", f=F),
                    inp[g * group:(g + 1) * group].rearrange("s j p f -> p (s j) f"),
                ).then_inc(dma_sems[slot], 16)
            sync.wait_ge(act_sem, 2 * total_groups * group)
            sync.wait_ge(dve_sem, total_groups * group)
            sync.dma_start(out_acc, acc[:]).then_inc(out_sem, 16)
            sync.wait_ge(out_sem, 16)

        @block.scalar
        def _(scalar):
            for t in range(total_groups):
                g = t % ng
                slot = t % IN_BUFS
                sslot = t % SIG_BUFS
                scalar.wait_ge(dma_sems[slot], 16 * (t // IN_BUFS + 1))
                if t >= SIG_BUFS:
                    # DVE must be done reading sig of the group in this slot
                    scalar.wait_ge(dve_sem, (t - SIG_BUFS + 1) * group)
                for s in range(group):
                    tq = t * group + s       # absolute op counter
                    q = g * group + s        # data/accum column
                    c = slice(s * F, (s + 1) * F)
                    cp = slice((2 * s) * F, (2 * s + 1) * F)      # predict
                    ct = slice((2 * s + 1) * F, (2 * s + 2) * F)  # target
                    nc.scalar.activation(
                        sig_buf[:, sslot, c], in_buf[:, slot, cp], AF.Sigmoid,
                        accum_out=sp_acc[:, q:q + 1],
                    ).then_inc(act_sem, 1)
                    if tq >= 2:
                        # scr_a slot WAW vs copy(tq-2); already satisfied
                        scalar.wait_ge(act_sem, 2 * (tq - 1))
                    nc.scalar.activation(
                        scr_a[:, tq % 2, :], in_buf[:, slot, ct], AF.Copy,
                        accum_out=st_acc[:, q:q + 1],
                    ).then_inc(act_sem, 1)

        @block.vector
        def _(vector):
            for t in range(total_groups):
                g = t % ng
                slot = t % IN_BUFS
                sslot = t % SIG_BUFS
                for s in range(group):
                    tq = t * group + s
                    q = g * group + s
                    c = slice(s * F, (s + 1) * F)
                    ct = slice((2 * s + 1) * F, (2 * s + 2) * F)  # target
                    vector.wait_ge(act_sem, 2 * tq + 1)   # sigmoid(tq) done
                    if tq >= 2:
                        # scr_v slot WAW vs stt(tq-2); already satisfied
                        vector.wait_ge(dve_sem, tq - 1)
                    nc.vector.scalar_tensor_tensor(
                        out=scr_v[:, tq % 2, :],
                        in0=sig_buf[:, sslot, c],
                        scalar=1.0,
                        in1=in_buf[:, slot, ct],
                        op0=ALU.mult, op1=ALU.mult,
                        accum_out=spt_acc[:, q:q + 1],
                    ).then_inc(dve_sem, 1)

    return nc


_NC_CACHE = {}


def _get_nc():
    if "nc" not in _NC_CACHE:
        _NC_CACHE["nc"] = build_nc()
    return _NC_CACHE["nc"]


def shard_inputs(predict, target):
    pred_sh = np.ascontiguousarray(predict, dtype=np.float32).reshape(
        N_CORES, S, P, F)
    tgt_sh = np.ascontiguousarray(target, dtype=np.float32).reshape(
        N_CORES, S, P, F)
    return [
        {"inp": np.stack([pred_sh[i], tgt_sh[i]], axis=1)}
        for i in range(N_CORES)
    ]


def finish(results, target):
    """Host-side: partition-sum [128, 3, S] partials + dice math."""
    sp = np.empty((N_CORES, S), np.float64)
    st = np.empty((N_CORES, S), np.float64)
    spt = np.empty((N_CORES, S), np.float64)
    for i, r in enumerate(results):
        a = r["out_acc"].astype(np.float64).sum(axis=0)   # [3, S]
        sp[i], st[i], spt[i] = a[0], a[1], a[2]

    dice = 1.0 - 2.0 * spt / (sp + st + SMOOTH)          # [B*O, D]
    tfirst = target.reshape(B * O, D, HW)[:, :, 0]       # [B*O, D]
    valid = (tfirst != -1.0).astype(np.float64)
    per_pair = (dice * valid).sum(axis=-1) / valid.sum(axis=-1)  # [B*O]
    return np.array(per_pair.mean(), dtype=np.float32)


def kernel(predict: np.ndarray, target: np.ndarray) -> np.ndarray:
    predict = np.asarray(predict)
    target = np.asarray(target)
    assert predict.shape == (B, O, D, 256, 256)
    in_maps = shard_inputs(predict, target)
    nc = _get_nc()
    res = run_bass_kernel_spmd(nc, in_maps, list(range(N_CORES)))
    return finish(res.results, target)


# revision 17
# speedup vs baseline: 8.8188x; 1.1732x over previous
"""DiceLoss kernel for Trainium2 (raw Bass, no Tile), 8-core data parallel.

Problem: predict/target [2, 4, 64, 256, 256] f32.
  p = sigmoid(predict); per (b, o, d) slice of 65536 elements:
    num = sum(p*t), den = sum(p) + sum(t) + 1
    dice = 1 - 2*num/den
  per-(b,o) mean over valid d slices, then mean over the 8 (b,o) pairs.

Sharding: B*O = 8 (b, o) pairs, one per core; each core handles its
pair's 64 depth slices (16 MiB predict + 16 MiB target -> DMA-bound,
~94 us/core at ~358 GB/s). Host stacks predict+target per core into one
[2, 64, 128, 512] array so each group load is a single 4 MiB dma_start.

Per slice (viewed [128 x 512]) on each core:
  ScalarE: sigmoid(predict) -> sig, accum_out -> sum(p) column
  VectorE: scalar_tensor_tensor (sig*1.0)*target -> scratch,
           accum_out -> sum(p*t) column   (single fused op)
  sum(t):  split across the two engines so neither exceeds the DMA
           floor — odd slices on ScalarE (Copy + accum_out; Copy and
           Sigmoid share one ACT table -> no reloads), even slices on
           VectorE (tensor_scalar *1.0 + accum_out).
(tensor_tensor_reduce is not used: its InstISA encoding is rejected by
this walrus build. Tile is not used: its kernel-tail drain exceeds this
build's per-instruction sync-wait limit.)

Scratch outputs rotate over 2 slots so same-engine WAW conflicts sit at
distance 2; a wait_ge on the engine's own (already-passed) semaphore
value proves the ordering to the race detector without stalling.

The [128, 3, 64] accumulator tile is DMA'd out once; host does the
partition sum and the tiny dice math over 512 slices.
"""

from contextlib import ExitStack

import numpy as np

import concourse.bass as bass
from concourse import mybir
from concourse.bass_utils import run_bass_kernel_spmd

N_CORES = 8
B, O, D = 2, 4, 64
HW = 256 * 256          # elements per slice
P = 128                 # SBUF partitions
F = HW // P             # 512 free elems per partition per slice
S = (B * O * D) // N_CORES  # 64 slices per core (= one (b,o) pair)
G = 8                   # slices per DMA group (4 MiB per dma_start)
IN_BUFS = 3             # input group buffers
SIG_BUFS = 2            # sigmoid output group buffers
SMOOTH = 1.0

f32 = mybir.dt.float32
AF = mybir.ActivationFunctionType
ALU = mybir.AluOpType


def build_nc(n_slices=S, group=G, repeats=1):
    """Build the per-core Bass program (same program on all cores).

    repeats > 1 re-runs the whole body that many times (re-reading the
    same DRAM) — used only for slope-based wall-clock timing."""
    assert n_slices % group == 0
    ng = n_slices // group
    total_groups = ng * repeats
    nc = bass.Bass("TRN2", debug=False, enable_asserts=False)

    # inp[s, 0] = predict slice s, inp[s, 1] = target slice s (host-stacked);
    # the (s, 2) dims merge into one stride run so a group load is a 3-dim AP.
    inp = nc.dram_tensor("inp", [n_slices, 2, P, F], f32, kind="ExternalInput").ap()
    # out_acc[:, 0] = sum(p), out_acc[:, 1] = sum(t), out_acc[:, 2] = sum(p*t)
    out_acc = nc.dram_tensor("out_acc", [P, 3, n_slices], f32,
                             kind="ExternalOutput").ap()

    with ExitStack() as ctx:
        # in_buf slot layout: m = 2*s + j blocks of F: predict slice s at
        # m=2s, target slice s at m=2s+1
        in_buf = ctx.enter_context(nc.sbuf_tensor([P, IN_BUFS, 2 * group * F], f32))
        sig_buf = ctx.enter_context(nc.sbuf_tensor([P, SIG_BUFS, group * F], f32))
        scr_a = ctx.enter_context(nc.sbuf_tensor([P, 2, F], f32))   # ACT copy dest
        scr_v = ctx.enter_context(nc.sbuf_tensor([P, 2, F], f32))   # DVE stt dest
        scr_t = ctx.enter_context(nc.sbuf_tensor([P, 2, F], f32))   # DVE ts dest
        acc = ctx.enter_context(nc.sbuf_tensor([P, 3, n_slices], f32))
        # One DMA sem per input slot: at most one group's load in flight per
        # sem, so "sem >= 16*uses" proves the load is complete. A single
        # cumulative sem would be unsound with >1 DMA in flight (fast SDMA
        # engines can contribute shards of later DMAs to the count).
        dma_sems = [
            ctx.enter_context(nc.semaphore(f"dma_sem{i}")) for i in range(IN_BUFS)
        ]
        out_sem = ctx.enter_context(nc.semaphore("out_sem"))
        sig_sem = ctx.enter_context(nc.semaphore("sig_sem"))    # +1 per sigmoid
        actc_sem = ctx.enter_context(nc.semaphore("actc_sem"))  # +1 per ACT copy
        dve_sem = ctx.enter_context(nc.semaphore("dve_sem"))    # +1 per slice
        block = ctx.enter_context(nc.Block())

        sp_acc = acc[:, 0, :]
        st_acc = acc[:, 1, :]
        spt_acc = acc[:, 2, :]
        copies_per_group = sum(1 for s in range(group) if s % 2 == 1)

        @block.sync
        def _(sync):
            for t in range(total_groups):
                g = t % ng
                slot = t % IN_BUFS
                if t >= IN_BUFS:
                    done = t - IN_BUFS + 1   # groups fully consumed
                    sync.wait_ge(sig_sem, done * group)
                    sync.wait_ge(actc_sem, done * copies_per_group)
                    sync.wait_ge(dve_sem, done * group)
                sync.dma_start(
                    in_buf[:, slot, :].rearrange("p (m f) -> p m f", f=F),
                    inp[g * group:(g + 1) * group].rearrange("s j p f -> p (s j) f"),
                ).then_inc(dma_sems[slot], 16)
            sync.wait_ge(sig_sem, total_groups * group)
            sync.wait_ge(actc_sem, total_groups * copies_per_group)
            sync.wait_ge(dve_sem, total_groups * group)
            sync.dma_start(out_acc, acc[:]).then_inc(out_sem, 16)
            sync.wait_ge(out_sem, 16)

        @block.scalar
        def _(scalar):
            cidx = 0  # running ACT-copy counter
            for t in range(total_groups):
                g = t % ng
                slot = t % IN_BUFS
                sslot = t % SIG_BUFS
                scalar.wait_ge(dma_sems[slot], 16 * (t // IN_BUFS + 1))
                if t >= SIG_BUFS:
                    # DVE must be done reading sig of the group in this slot
                    scalar.wait_ge(dve_sem, (t - SIG_BUFS + 1) * group)
                for s in range(group):
                    q = g * group + s        # data/accum column
                    c = slice(s * F, (s + 1) * F)
                    cp = slice((2 * s) * F, (2 * s + 1) * F)      # predict
                    ct = slice((2 * s + 1) * F, (2 * s + 2) * F)  # target
                    nc.scalar.activation(
                        sig_buf[:, sslot, c], in_buf[:, slot, cp], AF.Sigmoid,
                        accum_out=sp_acc[:, q:q + 1],
                    ).then_inc(sig_sem, 1)
                    if s % 2 == 1:
                        if cidx >= 2:
                            # scr_a slot WAW vs copy cidx-2; already satisfied
                            scalar.wait_ge(actc_sem, cidx - 1)
                        nc.scalar.activation(
                            scr_a[:, cidx % 2, :], in_buf[:, slot, ct], AF.Copy,
                            accum_out=st_acc[:, q:q + 1],
                        ).then_inc(actc_sem, 1)
                        cidx += 1

        @block.vector
        def _(vector):
            for t in range(total_groups):
                g = t % ng
                slot = t % IN_BUFS
                sslot = t % SIG_BUFS
                for s in range(group):
                    tq = t * group + s       # absolute slice counter
                    q = g * group + s
                    c = slice(s * F, (s + 1) * F)
                    ct = slice((2 * s + 1) * F, (2 * s + 2) * F)  # target
                    vector.wait_ge(sig_sem, tq + 1)   # sigmoid(tq) done
                    if tq >= 2:
                        # scr_v/scr_t slot WAW vs ops of slice tq-2;
                        # already satisfied
                        vector.wait_ge(dve_sem, tq - 1)
                    stt = nc.vector.scalar_tensor_tensor(
                        out=scr_v[:, tq % 2, :],
                        in0=sig_buf[:, sslot, c],
                        scalar=1.0,
                        in1=in_buf[:, slot, ct],
                        op0=ALU.mult, op1=ALU.mult,
                        accum_out=spt_acc[:, q:q + 1],
                    )
                    if s % 2 == 0:
                        nc.vector.tensor_scalar(
                            out=scr_t[:, tq % 2, :], in0=in_buf[:, slot, ct],
                            scalar1=1.0, scalar2=None,
                            op0=ALU.mult, op1=ALU.add,
                            accum_out=st_acc[:, q:q + 1],
                        ).then_inc(dve_sem, 1)
                    else:
                        stt.then_inc(dve_sem, 1)

    return nc


_NC_CACHE = {}


def _get_nc():
    if "nc" not in _NC_CACHE:
        _NC_CACHE["nc"] = build_nc()
    return _NC_CACHE["nc"]


def shard_inputs(predict, target):
    pred_sh = np.ascontiguousarray(predict, dtype=np.float32).reshape(
        N_CORES, S, P, F)
    tgt_sh = np.ascontiguousarray(target, dtype=np.float32).reshape(
        N_CORES, S, P, F)
    return [
        {"inp": np.stack([pred_sh[i], tgt_sh[i]], axis=1)}
        for i in range(N_CORES)
    ]


def finish(results, target):
    """Host-side: partition-sum [128, 3, S] partials + dice math."""
    sp = np.empty((N_CORES, S), np.float64)
    st = np.empty((N_CORES, S), np.float64)
    spt = np.empty((N_CORES, S), np.float64)
    for i, r in enumerate(results):
        a = r["out_acc"].astype(np.float64).sum(axis=0)   # [3, S]
        sp[i], st[i], spt[i] = a[0], a[1], a[2]

    dice = 1.0 - 2.0 * spt / (sp + st + SMOOTH)          # [B*O, D]
    tfirst = target.reshape(B * O, D, HW)[:, :, 0]       # [B*O, D]
    valid = (tfirst != -1.0).astype(np.float64)
    per_pair = (dice * valid).sum(axis=-1) / valid.sum(axis=-1)  # [B*O]
    return np.array(per_pair.mean(), dtype=np.float32)


def kernel(predict: np.ndarray, target: np.ndarray) -> np.ndarray:
    predict = np.asarray(predict)
    target = np.asarray(target)
    assert predict.shape == (B, O, D, 256, 256)
    in_maps = shard_inputs(predict, target)
    nc = _get_nc()
    res = run_bass_kernel_spmd(nc, in_maps, list(range(N_CORES)))
    return finish(res.results, target)


# revision 18
# speedup vs baseline: 10.3088x; 1.1690x over previous
"""DiceLoss kernel for Trainium2 (raw Bass, no Tile), 8-core data parallel.

Problem: predict/target [2, 4, 64, 256, 256] f32.
  p = sigmoid(predict); per (b, o, d) slice of 65536 elements:
    num = sum(p*t), den = sum(p) + sum(t) + 1
    dice = 1 - 2*num/den
  per-(b,o) mean over valid d slices, then mean over the 8 (b,o) pairs.

Sharding: B*O = 8 (b, o) pairs, one per core; each core handles its
pair's 64 depth slices (16 MiB predict + 16 MiB target -> DMA-bound,
~78 us/core measured, ~429 GB/s effective). Host interleaves predict and
target per slice into one [64, 2, 128, 512] array per core so each
group load is a single 4 MiB dma_start.

Per slice (viewed [128 x 512]) on each core:
  ScalarE: sigmoid(predict) -> sig, accum_out -> sum(p) column
  VectorE: scalar_tensor_tensor (sig*1.0)*target -> scratch,
           accum_out -> sum(p*t) column   (single fused op)
  sum(t):  split across the two engines so neither exceeds the DMA
           floor — odd slices on ScalarE (Copy + accum_out; Copy and
           Sigmoid share one ACT table -> no reloads), even slices on
           VectorE (tensor_scalar *1.0 + accum_out).
(tensor_tensor_reduce is not used: its InstISA encoding is rejected by
this walrus build. Tile is not used: its kernel-tail drain exceeds this
build's per-instruction sync-wait limit.)

Scratch outputs rotate over 2 slots so same-engine WAW conflicts sit at
distance 2; a wait_ge on the engine's own (already-passed) semaphore
value proves the ordering to the race detector without stalling.

The [128, 3, 64] accumulator tile is DMA'd out once; host does the
partition sum and the tiny dice math over 512 slices.
"""

from contextlib import ExitStack

import numpy as np

import concourse.bass as bass
from concourse import mybir
from concourse.bass_utils import run_bass_kernel_spmd

N_CORES = 8
B, O, D = 2, 4, 64
HW = 256 * 256          # elements per slice
P = 128                 # SBUF partitions
F = HW // P             # 512 free elems per partition per slice
S = (B * O * D) // N_CORES  # 64 slices per core (= one (b,o) pair)
G = 8                   # slices per DMA group (4 MiB per dma_start)
IN_BUFS = 3             # input group buffers
SIG_BUFS = 2            # sigmoid output group buffers
SMOOTH = 1.0

f32 = mybir.dt.float32
AF = mybir.ActivationFunctionType
ALU = mybir.AluOpType


def build_nc(n_slices=S, group=G, repeats=1):
    """Build the per-core Bass program (same program on all cores).

    repeats > 1 re-runs the whole body that many times (re-reading the
    same DRAM) — used only for slope-based wall-clock timing."""
    assert n_slices % group == 0
    ng = n_slices // group
    total_groups = ng * repeats
    nc = bass.Bass("TRN2", debug=False, enable_asserts=False)

    # inp[s, 0] = predict slice s, inp[s, 1] = target slice s (host-stacked);
    # the (s, 2) dims merge into one stride run so a group load is a 3-dim AP.
    inp = nc.dram_tensor("inp", [n_slices, 2, P, F], f32, kind="ExternalInput").ap()
    # out_acc[:, 0] = sum(p), out_acc[:, 1] = sum(t), out_acc[:, 2] = sum(p*t)
    out_acc = nc.dram_tensor("out_acc", [P, 3, n_slices], f32,
                             kind="ExternalOutput").ap()

    with ExitStack() as ctx:
        # in_buf slot layout: m = 2*s + j blocks of F: predict slice s at
        # m=2s, target slice s at m=2s+1
        in_buf = ctx.enter_context(nc.sbuf_tensor([P, IN_BUFS, 2 * group * F], f32))
        sig_buf = ctx.enter_context(nc.sbuf_tensor([P, SIG_BUFS, group * F], f32))
        scr_a = ctx.enter_context(nc.sbuf_tensor([P, 2, F], f32))   # ACT copy dest
        scr_v = ctx.enter_context(nc.sbuf_tensor([P, 2, F], f32))   # DVE stt dest
        scr_t = ctx.enter_context(nc.sbuf_tensor([P, 2, F], f32))   # DVE ts dest
        acc = ctx.enter_context(nc.sbuf_tensor([P, 3, n_slices], f32))
        # One DMA sem per input slot: at most one group's load in flight per
        # sem, so "sem >= 16*uses" proves the load is complete. A single
        # cumulative sem would be unsound with >1 DMA in flight (fast SDMA
        # engines can contribute shards of later DMAs to the count).
        dma_sems = [
            ctx.enter_context(nc.semaphore(f"dma_sem{i}")) for i in range(IN_BUFS)
        ]
        out_sem = ctx.enter_context(nc.semaphore("out_sem"))
        sig_sem = ctx.enter_context(nc.semaphore("sig_sem"))    # +1 per sigmoid
        actc_sem = ctx.enter_context(nc.semaphore("actc_sem"))  # +1 per ACT copy
        dve_sem = ctx.enter_context(nc.semaphore("dve_sem"))    # +1 per slice
        block = ctx.enter_context(nc.Block())

        sp_acc = acc[:, 0, :]
        st_acc = acc[:, 1, :]
        spt_acc = acc[:, 2, :]
        copies_per_group = sum(1 for s in range(group) if s % 2 == 1)

        @block.sync
        def _(sync):
            for t in range(total_groups):
                g = t % ng
                slot = t % IN_BUFS
                if t >= IN_BUFS:
                    done = t - IN_BUFS + 1   # groups fully consumed
                    sync.wait_ge(sig_sem, done * group)
                    sync.wait_ge(actc_sem, done * copies_per_group)
                    sync.wait_ge(dve_sem, done * group)
                sync.dma_start(
                    in_buf[:, slot, :].rearrange("p (m f) -> p m f", f=F),
                    inp[g * group:(g + 1) * group].rearrange("s j p f -> p (s j) f"),
                ).then_inc(dma_sems[slot], 16)
            sync.wait_ge(sig_sem, total_groups * group)
            sync.wait_ge(actc_sem, total_groups * copies_per_group)
            sync.wait_ge(dve_sem, total_groups * group)
            sync.dma_start(out_acc, acc[:]).then_inc(out_sem, 16)
            sync.wait_ge(out_sem, 16)

        @block.scalar
        def _(scalar):
            cidx = 0  # running ACT-copy counter
            for t in range(total_groups):
                g = t % ng
                slot = t % IN_BUFS
                sslot = t % SIG_BUFS
                scalar.wait_ge(dma_sems[slot], 16 * (t // IN_BUFS + 1))
                if t >= SIG_BUFS:
                    # DVE must be done reading sig of the group in this slot
                    scalar.wait_ge(dve_sem, (t - SIG_BUFS + 1) * group)
                for s in range(group):
                    q = g * group + s        # data/accum column
                    c = slice(s * F, (s + 1) * F)
                    cp = slice((2 * s) * F, (2 * s + 1) * F)      # predict
                    ct = slice((2 * s + 1) * F, (2 * s + 2) * F)  # target
                    nc.scalar.activation(
                        sig_buf[:, sslot, c], in_buf[:, slot, cp], AF.Sigmoid,
                        accum_out=sp_acc[:, q:q + 1],
                    ).then_inc(sig_sem, 1)
                    if s % 2 == 1:
                        if cidx >= 2:
                            # scr_a slot WAW vs copy cidx-2; already satisfied
                            scalar.wait_ge(actc_sem, cidx - 1)
                        nc.scalar.activation(
                            scr_a[:, cidx % 2, :], in_buf[:, slot, ct], AF.Copy,
                            accum_out=st_acc[:, q:q + 1],
                        ).then_inc(actc_sem, 1)
                        cidx += 1

        @block.vector
        def _(vector):
            for t in range(total_groups):
                g = t % ng
                slot = t % IN_BUFS
                sslot = t % SIG_BUFS
                for s in range(group):
                    tq = t * group + s       # absolute slice counter
                    q = g * group + s
                    c = slice(s * F, (s + 1) * F)
                    ct = slice((2 * s + 1) * F, (2 * s + 2) * F)  # target
                    vector.wait_ge(sig_sem, tq + 1)   # sigmoid(tq) done
                    if tq >= 2:
                        # scr_v/scr_t slot WAW vs ops of slice tq-2;
                        # already satisfied
                        vector.wait_ge(dve_sem, tq - 1)
                    stt = nc.vector.scalar_tensor_tensor(
                        out=scr_v[:, tq % 2, :],
                        in0=sig_buf[:, sslot, c],
                        scalar=1.0,
                        in1=in_buf[:, slot, ct],
                        op0=ALU.mult, op1=ALU.mult,
                        accum_out=spt_acc[:, q:q + 1],
                    )
                    if s % 2 == 0:
                        nc.vector.tensor_scalar(
                            out=scr_t[:, tq % 2, :], in0=in_buf[:, slot, ct],
                            scalar1=1.0, scalar2=None,
                            op0=ALU.mult, op1=ALU.add,
                            accum_out=st_acc[:, q:q + 1],
                        ).then_inc(dve_sem, 1)
                    else:
                        stt.then_inc(dve_sem, 1)

    return nc


_NC_CACHE = {}


def _get_nc():
    if "nc" not in _NC_CACHE:
        _NC_CACHE["nc"] = build_nc()
    return _NC_CACHE["nc"]


def shard_inputs(predict, target):
    pred_sh = np.ascontiguousarray(predict, dtype=np.float32).reshape(
        N_CORES, S, P, F)
    tgt_sh = np.ascontiguousarray(target, dtype=np.float32).reshape(
        N_CORES, S, P, F)
    return [
        {"inp": np.stack([pred_sh[i], tgt_sh[i]], axis=1)}
        for i in range(N_CORES)
    ]


def finish(results, target):
    """Host-side: partition-sum [128, 3, S] partials + dice math."""
    sp = np.empty((N_CORES, S), np.float64)
    st = np.empty((N_CORES, S), np.float64)
    spt = np.empty((N_CORES, S), np.float64)
    for i, r in enumerate(results):
        a = r["out_acc"].astype(np.float64).sum(axis=0)   # [3, S]
        sp[i], st[i], spt[i] = a[0], a[1], a[2]

    dice = 1.0 - 2.0 * spt / (sp + st + SMOOTH)          # [B*O, D]
    tfirst = target.reshape(B * O, D, HW)[:, :, 0]       # [B*O, D]
    valid = (tfirst != -1.0).astype(np.float64)
    per_pair = (dice * valid).sum(axis=-1) / valid.sum(axis=-1)  # [B*O]
    return np.array(per_pair.mean(), dtype=np.float32)


def kernel(predict: np.ndarray, target: np.ndarray) -> np.ndarray:
    predict = np.asarray(predict)
    target = np.asarray(target)
    assert predict.shape == (B, O, D, 256, 256)
    in_maps = shard_inputs(predict, target)
    nc = _get_nc()
    res = run_bass_kernel_spmd(nc, in_maps, list(range(N_CORES)))
    return finish(res.results, target)
